# revision 12
# baseline (speedup 1.0000x reference)
"""InterpretableMultiHeadAttention Trainium2 kernel.

Full-input contract: kernel(**inputs) takes the complete tensors and returns
(output, attn_weights) exactly like the reference nn.Module.

Sharding: data-parallel over (batch=2) x (4 query blocks of 512) = 8 cores.
No collectives needed: softmax is over k (intact per core) and the head-mean
is over heads (all resident per core).

Device-side layout is fully transposed ([feature-on-partitions, seq-on-free]):
  - host pre-transposes query/key/value/weights so all DMAs are contiguous
  - scores^T[k,q] = (K^T-slice).T @ Q^T-slice per head (C=64, row-packed pairs)
  - exp on ScalarE from PSUM; denominator via ones-column appended to V
    (context matmul row 64 accumulates sum_k exp for free)
  - per-head context scaled by 1/s and averaged into ctx_accum (VectorE)
  - attention written back as attnT[h,k,q]; host transposes to [h,q,k]

Matmuls run in float32r (TF32-like input rounding, fp32 accumulate, 4x the
fp32 matmul rate). walrus requires every f32r matmul operand's producer to
emit f32r: DMA-loaded operands are declared f32r at the DRAM level; computed
operands (K^T/Q^T/Vaug/exp/ctx) are written as f32r by their PSUM-evict or
activation instruction.
"""

import os
import numpy as np
from contextlib import ExitStack
from concurrent.futures import ThreadPoolExecutor

import concourse.mybir as mybir
import concourse.tile as tile
from concourse import bacc
from concourse import bass_utils

# Problem dims (hardcoded per contract)
B, QL, KL, HIDDEN, HEADS, D = 2, 2048, 2048, 1024, 16, 64
NCORES = 8
QBLK = QL * B // NCORES  # 512 query rows per core
P = 128
F32 = mybir.dt.float32
F32R = mybir.dt.float32r

_COMPILED = {}


def build_nc(hidden=HIDDEN, kl=KL, heads=HEADS, qblk=QBLK, mm_fast=True):
    """Build + compile the per-core Bass program. All 8 cores run the same
    program on different data."""
    C = hidden // P          # contraction tiles over hidden
    KC = kl // P             # k chunks of 128
    NQ = max(qblk // 512, 1)
    QF = min(qblk, 512)      # q free-dim chunk for matmuls
    assert qblk % QF == 0
    T = heads // 2           # head pairs
    mdt = F32R if mm_fast else F32

    nc = bacc.Bacc("TRN2", debug=False, num_devices=NCORES)

    qT = nc.dram_tensor("qT", [hidden, qblk], mdt, kind="ExternalInput").ap()
    kT = nc.dram_tensor("kT", [hidden, kl], mdt, kind="ExternalInput").ap()
    vT = nc.dram_tensor("vT", [hidden, kl], mdt, kind="ExternalInput").ap()
    wqT = nc.dram_tensor("wqT", [hidden, hidden], mdt, kind="ExternalInput").ap()
    wkT = nc.dram_tensor("wkT", [hidden, hidden], mdt, kind="ExternalInput").ap()
    wvT = nc.dram_tensor("wvT", [hidden, D], mdt, kind="ExternalInput").ap()
    woT = nc.dram_tensor("woT", [D, hidden], mdt, kind="ExternalInput").ap()
    bo = nc.dram_tensor("bo", [P, hidden // P], F32, kind="ExternalInput").ap()

    attnT = nc.dram_tensor("attnT", [heads, kl, qblk], F32, kind="ExternalOutput").ap()
    outT = nc.dram_tensor("outT", [hidden, qblk], F32, kind="ExternalOutput").ap()

    scale = float(1.0 / np.sqrt(D))

    with tile.TileContext(nc) as tc:
        with ExitStack() as stack:
            def pool(name, bufs, space="SBUF"):
                return stack.enter_context(
                    tc.tile_pool(name=name, bufs=bufs, space=space))

            hin = pool("hin", C)            # vt then kt tiles
            qin = pool("qin", C)
            wpool = pool("wpool", C)        # wq then wk tiles
            wvp = pool("wvp", C)
            wop = pool("wop", 1)
            ktp = pool("ktp", 2)
            qtp = pool("qtp", C)
            vaugp = pool("vaug", KC)
            # all KC exp tiles of a head stay live until its normalize pass
            expp = pool("expp", KC + 2)
            repp = pool("repp", 2)
            rsp = pool("rsp", 1)
            ctxp = pool("ctxp", 2)
            constp = pool("constp", 1)
            outsbp = pool("outsb", 1)
            ps_scores = pool("ps", 3, space="PSUM")
            ps_ctx = pool("pc", 2, space="PSUM")
            ps_proj = pool("pp", 2, space="PSUM")
            ps_rep = pool("pr", 1, space="PSUM")

            # ---- constants ----
            ones = constp.tile([P, P], F32)
            nc.vector.memset(ones[:], 1.0)
            bo_sb = constp.tile([P, hidden // P], F32)
            nc.sync.dma_start(bo_sb[:], bo)
            wo_sb = wop.tile([D, hidden], mdt)
            nc.sync.dma_start(wo_sb[:], woT)

            # ---- V projection -> Vaug[kc] = [V[k,:64] | ones] ----
            wv_sb = []
            for c in range(C):
                w = wvp.tile([P, D], mdt)
                nc.sync.dma_start(w[:], wvT[c * P:(c + 1) * P, :])
                wv_sb.append(w)
            vt_sb = []
            for c in range(C):
                t_ = hin.tile([P, kl], mdt, tag="hin")
                nc.sync.dma_start(t_[:], vT[c * P:(c + 1) * P, :])
                vt_sb.append(t_)
            vaug = []
            for kc in range(KC):
                pv = ps_proj.tile([P, D], F32, tag="pp")
                for c in range(C):
                    nc.tensor.matmul(
                        pv[:], vt_sb[c][:, kc * P:(kc + 1) * P], wv_sb[c][:],
                        start=(c == 0), stop=(c == C - 1))
                va = vaugp.tile([P, D + 1], mdt)
                nc.scalar.copy(va[:, 0:D], pv[:])
                nc.vector.tensor_copy(va[:, D:D + 1], ones[:, 0:1])
                vaug.append(va)

            # ---- Q projection (scaled by 1/sqrt(D)) -> QT tiles [128, qblk] ----
            qt_in = []
            for c in range(C):
                t_ = qin.tile([P, qblk], mdt, tag="qin")
                nc.sync.dma_start(t_[:], qT[c * P:(c + 1) * P, :])
                qt_in.append(t_)
            wq_sb = []
            for c in range(C):
                w = wpool.tile([P, hidden], mdt, tag="w")
                nc.sync.dma_start(w[:], wqT[c * P:(c + 1) * P, :])
                wq_sb.append(w)
            QT = []
            for t in range(C):
                qtile = qtp.tile([P, qblk], mdt)
                for n in range(NQ):
                    pq = ps_proj.tile([P, QF], F32, tag="pp")
                    for c in range(C):
                        nc.tensor.matmul(
                            pq[:],
                            wq_sb[c][:, t * P:(t + 1) * P],
                            qt_in[c][:, n * QF:(n + 1) * QF],
                            start=(c == 0), stop=(c == C - 1))
                    nc.scalar.activation(
                        qtile[:, n * QF:(n + 1) * QF], pq[:],
                        mybir.ActivationFunctionType.Copy, scale=scale)
                QT.append(qtile)

            # ---- K weights (reuse wq slots) + key inputs (reuse vt slots) ----
            wk_sb = []
            for c in range(C):
                w = wpool.tile([P, hidden], mdt, tag="w")
                nc.sync.dma_start(w[:], wkT[c * P:(c + 1) * P, :])
                wk_sb.append(w)
            kt_in = []
            for c in range(C):
                t_ = hin.tile([P, kl], mdt, tag="hin")
                nc.sync.dma_start(t_[:], kT[c * P:(c + 1) * P, :])
                kt_in.append(t_)

            ctx_accum = constp.tile([D, QF * NQ], F32)
            NKF = kl // 512  # kproj free chunks

            # ---- per head-pair: K^T tile t, then attention for heads 2t,2t+1 ----
            for t in range(max(T, 1)):
                ktile = ktp.tile([P, kl], mdt)
                for n in range(NKF):
                    pk = ps_proj.tile([P, 512], F32, tag="pp")
                    for c in range(C):
                        nc.tensor.matmul(
                            pk[:],
                            wk_sb[c][:, t * P:(t + 1) * P],
                            kt_in[c][:, n * 512:(n + 1) * 512],
                            start=(c == 0), stop=(c == C - 1))
                    nc.scalar.copy(ktile[:, n * 512:(n + 1) * 512], pk[:])

                for hh in range(2):
                    h = 2 * t + hh
                    if h >= heads:
                        break
                    po = hh * 64
                    for n in range(NQ):
                        exp_tiles = []
                        pctx = ps_ctx.tile([D + 1, QF], F32, tag="pc")
                        for kc in range(KC):
                            pscore = ps_scores.tile([P, QF], F32, tag="ps")
                            nc.tensor.matmul(
                                pscore[:],
                                ktile[po:po + 64, kc * P:(kc + 1) * P],
                                QT[t][po:po + 64, n * QF:(n + 1) * QF],
                                start=True, stop=True)
                            et = expp.tile([P, QF], mdt, tag="exp")
                            nc.scalar.activation(
                                et[:], pscore[:], mybir.ActivationFunctionType.Exp)
                            exp_tiles.append(et)
                            nc.tensor.matmul(
                                pctx[:], vaug[kc][:], et[:],
                                start=(kc == 0), stop=(kc == KC - 1))
                        # 1/s lives in row 64 of pctx (the ones-column output)
                        rs = rsp.tile([P, QF], F32, tag="rs")
                        nc.vector.reciprocal(rs[64:65, :], pctx[D:D + 1, :])
                        prep = ps_rep.tile([P, QF], F32, tag="pr")
                        nc.tensor.matmul(
                            prep[:], ones[64:65, :], rs[64:65, :],
                            start=True, stop=True)
                        rep = repp.tile([P, QF], F32, tag="rep")
                        nc.scalar.copy(rep[:], prep[:])
                        # ctx_h = pctx[0:64] * rep ; accumulate over heads
                        nsl = slice(n * QF, (n + 1) * QF)
                        if h == 0:
                            nc.vector.tensor_tensor(
                                ctx_accum[:, nsl], pctx[0:D, :], rep[0:D, :],
                                op=mybir.AluOpType.mult)
                        else:
                            ctmp = ctxp.tile([D, QF], F32, tag="ctmp")
                            nc.vector.tensor_tensor(
                                ctmp[:], pctx[0:D, :], rep[0:D, :],
                                op=mybir.AluOpType.mult)
                            nc.vector.tensor_tensor(
                                ctx_accum[:, nsl], ctx_accum[:, nsl], ctmp[:],
                                op=mybir.AluOpType.add)
                        # normalize exp tiles in place and store (all writers
                        # of this memloc must emit f32r for the ctx matmul)
                        for kc in range(KC):
                            et = exp_tiles[kc]
                            nc.vector.tensor_tensor(
                                et[:], et[:], rep[:], op=mybir.AluOpType.mult)
                            nc.sync.dma_start(
                                attnT[h, kc * P:(kc + 1) * P, n * QF:(n + 1) * QF],
                                et.bitcast(F32)[:])

            # ---- output projection: outT = (w_out.T/16) rows @ ctx + b ----
            ctx_r = constp.tile([D, QF * NQ], mdt)
            nc.vector.tensor_copy(ctx_r[:], ctx_accum[:])
            for mo in range(C):
                for n in range(NQ):
                    pout = ps_proj.tile([P, QF], F32, tag="pp")
                    nc.tensor.matmul(
                        pout[:],
                        wo_sb[:, mo * P:(mo + 1) * P],
                        ctx_r[:, n * QF:(n + 1) * QF],
                        start=True, stop=True)
                    osb = outsbp.tile([P, QF], F32, tag="osb")
                    nc.scalar.activation(
                        osb[:], pout[:], mybir.ActivationFunctionType.Identity,
                        bias=bo_sb[:, mo:mo + 1])
                    nc.sync.dma_start(
                        outT[mo * P:(mo + 1) * P, n * QF:(n + 1) * QF], osb[:])

    nc.compile()
    return nc


def _get_nc():
    key = "full"
    if key not in _COMPILED:
        _COMPILED[key] = build_nc()
    return _COMPILED[key]


def make_in_maps(query, key, value, w_q, w_k, w_v, w_out, b_out):
    query = np.asarray(query, np.float32)
    key = np.asarray(key, np.float32)
    value = np.asarray(value, np.float32)
    wqT = np.ascontiguousarray(np.asarray(w_q, np.float32).T)
    wkT = np.ascontiguousarray(np.asarray(w_k, np.float32).T)
    wvT = np.ascontiguousarray(np.asarray(w_v, np.float32).T)
    woT = np.ascontiguousarray(np.asarray(w_out, np.float32).T) / np.float32(HEADS)
    bo = np.ascontiguousarray(
        np.asarray(b_out, np.float32).reshape(HIDDEN // P, P).T)
    kTs = [np.ascontiguousarray(key[b].T) for b in range(B)]
    vTs = [np.ascontiguousarray(value[b].T) for b in range(B)]
    in_maps = []
    for core in range(NCORES):
        b = core // (NCORES // B)
        qb = core % (NCORES // B)
        qs = slice(qb * QBLK, (qb + 1) * QBLK)
        in_maps.append({
            "qT": np.ascontiguousarray(query[b, qs, :].T),
            "kT": kTs[b],
            "vT": vTs[b],
            "wqT": wqT,
            "wkT": wkT,
            "wvT": wvT,
            "woT": woT,
            "bo": bo,
        })
    return in_maps


def assemble(results):
    output = np.empty((B, QL, HIDDEN), np.float32)
    attn = np.empty((B, HEADS, QL, KL), np.float32)

    def fill(job):
        core, h = job
        b = core // (NCORES // B)
        qb = core % (NCORES // B)
        qs = slice(qb * QBLK, (qb + 1) * QBLK)
        res = results[core]
        if h == 0:
            output[b, qs, :] = res["outT"].T
        attn[b, h, qs, :] = res["attnT"][h].T

    jobs = [(c, h) for c in range(NCORES) for h in range(HEADS)]
    with ThreadPoolExecutor(max_workers=16) as ex:
        list(ex.map(fill, jobs))
    return output, attn


def kernel(query, key, value, w_q, w_k, w_v, w_out, b_out):
    nc = _get_nc()
    in_maps = make_in_maps(query, key, value, w_q, w_k, w_v, w_out, b_out)
    res = bass_utils.run_bass_kernel_spmd(
        nc, in_maps, core_ids=list(range(NCORES)),
        trace=bool(os.environ.get("KERNEL_TRACE")))
    if os.environ.get("KERNEL_TRACE"):
        kernel.last_exec_time_ns = res.exec_time_ns
        kernel.last_trace = res.instructions_and_trace
    return assemble(res.results)
